# revision 26
# baseline (speedup 1.0000x reference)
"""Trainium2 Bass kernel for MHSA with Transformer-XL relative position bias.

Problem: B=16, T=1024, DM=256, H=4, HS=64 fp32.
Sharding: pure data-parallel over batch across 8 cores (2 batches/core).

Per-core pipeline (M = 2*1024 = 2048 rows):
  1. LN in [m, d] layout (bn_stats); xn PE-transposed -> xnT [256, M] bf16;
     pos cast to bf16 and xbar-transposed -> posT.
  2. Projections via PE: QuT/QvT/KT/PT [256, M] bf16 (s on partitions),
     V [M, 256] bf16.  QvT = QuT + (bqv-bqu) on DVE.
  3. Per (b, h) step pipeline (3 stages offset by one step):
     a) pos scores X = QvT.T @ PT -> PSUM -> bf16 -> DRAM scratch rows
        [T, T+1] (col 0 pre-zeroed once); written at col offset 1.
     b) rel-shift re-read (row stride T from element offset T); content
        scores C into PSUM; sheared R accumulated into the same PSUM via
        identity matmul; exp((C+R)/8) on ACT straight from PSUM with fused
        row-sum (logits small: no max subtraction); in-place A = E*(1/S);
        A transposed n<->m via xbar DMA into contiguous at4 layout.
     d) AV^T accumulated on PE from at4; evacuated to avT.
  4. Out-proj from avT, + bo + residual, DMA out.

DMA queues: scratch-write on gpsimd, scratch-read + xbar + io on sync.
"""
import sys

sys.path.insert(0, "/opt/trn_rl_repo")

import numpy as np

import concourse.bass as bass
import concourse.bacc as bacc
import concourse.tile as tile
from concourse import mybir
from concourse.masks import make_identity
from concourse.bass_utils import run_bass_kernel_spmd

B, T, DM, H, HS = 16, 1024, 256, 4, 64
NCORES = 8
BL = B // NCORES          # local batches per core
M = BL * T                # local rows (2048)
NMT = M // 128            # m-tiles (16)
P = 128
LN_EPS = 1e-3
F32 = mybir.dt.float32
BF16 = mybir.dt.bfloat16


def build_bass():
    nc = bacc.Bacc("TRN2", target_bir_lowering=False, debug=False,
                   enable_asserts=False, num_devices=NCORES)

    x_in = nc.dram_tensor("x", [M, DM], F32, kind="ExternalInput").ap()
    pos_in = nc.dram_tensor("pos", [M, DM], F32, kind="ExternalInput").ap()
    wq_in = nc.dram_tensor("wq", [DM, DM], F32, kind="ExternalInput").ap()
    wk_in = nc.dram_tensor("wk", [DM, DM], F32, kind="ExternalInput").ap()
    wv_in = nc.dram_tensor("wv", [DM, DM], F32, kind="ExternalInput").ap()
    wp_in = nc.dram_tensor("wp", [DM, DM], F32, kind="ExternalInput").ap()
    wo_in = nc.dram_tensor("wo", [DM, DM], F32, kind="ExternalInput").ap()
    bqu_in = nc.dram_tensor("bqu", [DM], F32, kind="ExternalInput").ap()
    bqv_in = nc.dram_tensor("bqv", [DM], F32, kind="ExternalInput").ap()
    bk_in = nc.dram_tensor("bk", [DM], F32, kind="ExternalInput").ap()
    bo_in = nc.dram_tensor("bo", [DM], F32, kind="ExternalInput").ap()
    out = nc.dram_tensor("out", [M, DM], F32, kind="ExternalOutput").ap()

    scr = [
        nc.dram_tensor(f"xscr{i}", [T, T + 1], BF16, kind="Internal").ap()
        for i in range(2)
    ]

    with tile.TileContext(nc) as tc:
        with tc.tile_pool(name="persist", bufs=1) as pp:
            # --- persistent SBUF ---
            ident = pp.tile([P, P], F32)
            make_identity(nc, ident)
            ident_bf = pp.tile([P, P], BF16)
            nc.vector.tensor_copy(out=ident_bf, in_=ident)

            def load_w(ap_in, name):
                # SWDGE cast-DMA: f32 DRAM -> bf16 SBUF directly
                ts = [pp.tile([P, DM], BF16, tag=f"{name}{c}", name=f"{name}{c}")
                      for c in range(2)]
                for c in range(2):
                    nc.gpsimd.dma_start(out=ts[c], in_=ap_in[c * P:(c + 1) * P, :])
                return ts

            # pos tiles on gpsimd, interleaved with weight cast-loads so both
            # streams arrive in time; zero-col scatter last (phase 3 only).
            pos_f32 = pp.tile([P, NMT, DM], F32, tag="pos_f32", name="pos_f32")

            def load_pos(ch):
                nc.gpsimd.dma_start(
                    out=pos_f32[:, 4 * ch:4 * ch + 4, :],
                    in_=bass.AP(tensor=pos_in.tensor, offset=4 * ch * P * DM,
                                ap=[[DM, P], [P * DM, 4], [1, DM]]),
                )

            load_pos(0)
            load_pos(1)
            wq_sb = load_w(wq_in, "wq")
            wk_sb = load_w(wk_in, "wk")
            wp_sb = load_w(wp_in, "wp")
            load_pos(2)
            wv_sb = load_w(wv_in, "wv")
            wo_sb = load_w(wo_in, "wo")
            load_pos(3)

            def load_col(ap_in, name):
                ts = [pp.tile([P, 1], F32, tag=f"{name}{c}", name=f"{name}{c}") for c in range(2)]
                for c in range(2):
                    nc.gpsimd.dma_start(
                        out=ts[c],
                        in_=bass.AP(tensor=ap_in.tensor, offset=c * P, ap=[[1, P], [1, 1]]),
                    )
                return ts

            bqu_c = load_col(bqu_in, "bqu")
            bqv_c = load_col(bqv_in, "bqv")
            bk_c = load_col(bk_in, "bk")
            dqv_c = [pp.tile([P, 1], F32, tag=f"dqv{c}", name=f"dqv{c}") for c in range(2)]
            for c in range(2):
                nc.vector.tensor_tensor(out=dqv_c[c], in0=bqv_c[c], in1=bqu_c[c],
                                        op=mybir.AluOpType.subtract)

            bo_b = pp.tile([P, DM], F32, tag="bo_b", name="bo_b")
            nc.gpsimd.dma_start(
                out=bo_b,
                in_=bass.AP(tensor=bo_in.tensor, offset=0, ap=[[0, P], [1, DM]]),
            )

            eps_t = pp.tile([P, 1], F32)
            nc.vector.memset(eps_t, LN_EPS)

            # zero column 0 of both scratch buffers (once; writes never touch it)
            zcol = pp.tile([P, 8], BF16, tag="zcol", name="zcol")
            nc.vector.memset(zcol, 0.0)
            for i in range(2):
                nc.gpsimd.dma_start(
                    out=bass.AP(tensor=scr[i].tensor, offset=0,
                                ap=[[T + 1, P], [P * (T + 1), 8]]),
                    in_=zcol,
                )

            x_res = pp.tile([P, NMT, DM], F32)        # residual copy of inputs
            xnT = [pp.tile([P, M], BF16, tag=f"xnT{c}", name=f"xnT{c}") for c in range(2)]
            pT = pp.tile([P, 2, M], BF16, tag="pT", name="pT")
            quT = [pp.tile([P, M], BF16, tag=f"quT{c}", name=f"quT{c}") for c in range(2)]
            qvT = [pp.tile([P, M], BF16, tag=f"qvT{c}", name=f"qvT{c}") for c in range(2)]
            kT = [pp.tile([P, M], BF16, tag=f"kT{c}", name=f"kT{c}") for c in range(2)]
            v_sb = pp.tile([P, NMT, DM], BF16)        # V[mt*128+p, s] at [:, mt, s]
            avT = [pp.tile([P, M], BF16, tag=f"avT{c}", name=f"avT{c}") for c in range(2)]

            # x loads in 4 chunks so LN can start after the first chunk
            for ch in range(4):
                nc.sync.dma_start(
                    out=x_res[:, 4 * ch:4 * ch + 4, :],
                    in_=bass.AP(tensor=x_in.tensor, offset=4 * ch * P * DM,
                                ap=[[DM, P], [P * DM, 4], [1, DM]]),
                )

            # ------- phases 1+2 merged per 4-mt chunk (PE stays dense) -----
            with tc.tile_pool(name="posp", bufs=1) as posp:
                posT = posp.tile([P, 2, M], BF16, tag="posT", name="posT")
                with tc.tile_pool(name="ph1", bufs=3) as sb1, \
                     tc.tile_pool(name="ps1", bufs=2, space="PSUM") as ps1, \
                     tc.tile_pool(name="ps2", bufs=2, space="PSUM") as ps2:
                    for ch in range(4):
                        for mt in range(4 * ch, 4 * ch + 4):
                            xs = x_res[:, mt, :]
                            stats = sb1.tile([P, 6], F32, tag="stats")
                            nc.vector.bn_stats(out=stats, in_=xs)
                            mv = sb1.tile([P, 2], F32, tag="mv")
                            nc.vector.bn_aggr(out=mv, in_=stats)
                            rstd = sb1.tile([P, 1], F32, tag="rstd")
                            nc.scalar.activation(out=rstd, in_=mv[:, 1:2],
                                                 func=mybir.ActivationFunctionType.Sqrt,
                                                 bias=eps_t, scale=1.0)
                            nc.vector.reciprocal(out=rstd, in_=rstd)
                            xn = sb1.tile([P, DM], F32, tag="xn")
                            nc.vector.tensor_scalar(out=xn, in0=xs, scalar1=mv[:, 0:1],
                                                    scalar2=rstd,
                                                    op0=mybir.AluOpType.subtract,
                                                    op1=mybir.AluOpType.mult)
                            # xn -> xnT via PE transpose, ACT evac
                            for c in range(2):
                                tp = ps1.tile([P, P], F32, tag="tp")
                                nc.tensor.transpose(tp, xn[:, c * P:(c + 1) * P], ident)
                                nc.scalar.copy(out=xnT[c][:, mt * P:(mt + 1) * P], in_=tp)
                            # pos: cast bf16 (DVE), xbar -> posT
                            ptb = sb1.tile([P, DM], BF16, tag="ptb")
                            nc.vector.tensor_copy(out=ptb, in_=pos_f32[:, mt, :])
                            nc.scalar.dma_start_transpose(
                                out=posT[:, :, mt * P:(mt + 1) * P], in_=ptb)
                        # projections for this chunk's m-range
                        msl = slice(ch * 512, (ch + 1) * 512)
                        for sc in range(2):
                            pq = ps2.tile([P, 512], F32, tag="pq")
                            pk = ps2.tile([P, 512], F32, tag="pk")
                            pps = ps2.tile([P, 512], F32, tag="pp")
                            for dc in range(2):
                                nc.tensor.matmul(pq, lhsT=wq_sb[dc][:, sc * P:(sc + 1) * P],
                                                 rhs=xnT[dc][:, msl],
                                                 start=(dc == 0), stop=(dc == 1))
                                nc.tensor.matmul(pk, lhsT=wk_sb[dc][:, sc * P:(sc + 1) * P],
                                                 rhs=xnT[dc][:, msl],
                                                 start=(dc == 0), stop=(dc == 1))
                                nc.tensor.matmul(pps, lhsT=wp_sb[dc][:, sc * P:(sc + 1) * P],
                                                 rhs=posT[:, dc, msl],
                                                 start=(dc == 0), stop=(dc == 1))
                            nc.scalar.activation(out=quT[sc][:, msl], in_=pq,
                                                 func=mybir.ActivationFunctionType.Identity,
                                                 bias=bqu_c[sc], scale=1.0)
                            nc.vector.tensor_scalar_add(out=qvT[sc][:, msl],
                                                        in0=quT[sc][:, msl],
                                                        scalar1=dqv_c[sc])
                            nc.scalar.activation(out=kT[sc][:, msl], in_=pk,
                                                 func=mybir.ActivationFunctionType.Identity,
                                                 bias=bk_c[sc], scale=1.0)
                            nc.vector.tensor_copy(out=pT[:, sc, msl], in_=pps)
                        for mt in range(4 * ch, 4 * ch + 4):
                            pv = ps2.tile([P, 512], F32, tag="pq")
                            for dc in range(2):
                                nc.tensor.matmul(pv[:, :DM],
                                                 lhsT=xnT[dc][:, mt * P:(mt + 1) * P],
                                                 rhs=wv_sb[dc],
                                                 start=(dc == 0), stop=(dc == 1))
                            nc.vector.tensor_copy(out=v_sb[:, mt, :], in_=pv[:, :DM])

            # ---------------- phase 3: attention per (b, h) ----------------
            with tc.tile_pool(name="ph3", bufs=5) as sb3, \
                 tc.tile_pool(name="abp", bufs=3) as abp, \
                 tc.tile_pool(name="at", bufs=2) as atp, \
                 tc.tile_pool(name="psA", bufs=2, space="PSUM") as psA, \
                 tc.tile_pool(name="psC", bufs=2, space="PSUM") as psC, \
                 tc.tile_pool(name="psAV", bufs=2, space="PSUM") as psAV:
                NBH = BL * H
                at_tiles = {}
                ab_tiles = {}
                rbf_tiles = {}

                xbf_tiles = {}

                def prefetch_rbf_pair(bh, q):
                    # rel-shift re-read for mt pair (2q, 2q+1), one DMA
                    sc_t = scr[bh % 2]
                    rbf2 = sb3.tile([P, 2, T], BF16, tag="rbf", name="rbf")
                    nc.gpsimd.dma_start(
                        out=rbf2,
                        in_=bass.AP(tensor=sc_t.tensor, offset=T + 2 * q * P * T,
                                    ap=[[T, P], [P * T, 2], [1, T]]))
                    rbf_tiles[(bh, q)] = rbf2

                def stage_a(bh, mt):
                    b, h = divmod(bh, H)
                    hh, po = h // 2, (h % 2) * 64
                    ssl = slice(po, po + 64)
                    sc_t = scr[bh % 2]
                    mg = slice(b * T + mt * P, b * T + (mt + 1) * P)
                    if mt % 2 == 0:
                        xbf_tiles[(bh, mt // 2)] = abp.tile([P, 2, T], BF16,
                                                            tag="xbf", name="xbf")
                    xbf2 = xbf_tiles[(bh, mt // 2)]
                    for nck in range(2):
                        xp = psA.tile([P, 512], F32, tag="xp", name="xp")
                        nc.tensor.matmul(
                            xp, lhsT=qvT[hh][ssl, mg],
                            rhs=pT[ssl, hh, b * T + nck * 512:b * T + (nck + 1) * 512],
                            start=True, stop=True)
                        osl = xbf2[:, mt % 2, nck * 512:(nck + 1) * 512]
                        if nck == 0 or mt % 2 == 1:
                            nc.vector.tensor_copy(out=osl, in_=xp)
                        else:
                            nc.scalar.copy(out=osl, in_=xp)
                    if mt % 2 == 1:
                        q = mt // 2
                        nc.gpsimd.dma_start(
                            out=bass.AP(tensor=sc_t.tensor,
                                        offset=2 * q * P * (T + 1) + 1,
                                        ap=[[T + 1, P], [P * (T + 1), 2], [1, T]]),
                            in_=xbf2)
                        del xbf_tiles[(bh, q)]
                        # issue the rel-shift reads as soon as the covering
                        # scratch writes are queued: R(q-1) needs w(q-1)+w(q);
                        # R(3) needs only w(3)
                        if q >= 1:
                            prefetch_rbf_pair(bh, q - 1)
                        if q == 3:
                            prefetch_rbf_pair(bh, 3)

                ssum_tiles = {}

                def stage_bc_mm(bh, mt):
                    b, h = divmod(bh, H)
                    hh, po = h // 2, (h % 2) * 64
                    ssl = slice(po, po + 64)
                    mg = slice(b * T + mt * P, b * T + (mt + 1) * P)
                    rbf2 = rbf_tiles[(bh, mt // 2)]
                    if mt % 2 == 1:
                        del rbf_tiles[(bh, mt // 2)]
                    cp = psC.tile([P, T], F32, tag="cp", name="cp")
                    for nck in range(2):
                        nc.tensor.matmul(
                            cp[:, nck * 512:(nck + 1) * 512], lhsT=quT[hh][ssl, mg],
                            rhs=kT[hh][ssl, b * T + nck * 512:b * T + (nck + 1) * 512],
                            start=True, stop=False)
                    for nck in range(2):
                        nc.tensor.matmul(
                            cp[:, nck * 512:(nck + 1) * 512], lhsT=ident_bf,
                            rhs=rbf2[:, mt % 2, nck * 512:(nck + 1) * 512],
                            start=False, stop=True)
                    if mt % 2 == 0:
                        ab_tiles[(bh, mt // 2)] = abp.tile([P, 2 * T], BF16,
                                                           tag="ab", name="ab")
                    abf2 = ab_tiles[(bh, mt // 2)]
                    half = abf2[:, (mt % 2) * T:(mt % 2) * T + T]
                    ssum = sb3.tile([P, 1], F32, tag="ssum", name="ssum")
                    nc.scalar.activation(out=half, in_=cp,
                                         func=mybir.ActivationFunctionType.Exp,
                                         scale=0.125, accum_out=ssum)
                    ssum_tiles[(bh, mt)] = ssum

                def stage_bc_fin(bh, mt):
                    # normalize (deferred one mt so exp has finished) + xbar
                    abf2 = ab_tiles[(bh, mt // 2)]
                    half = abf2[:, (mt % 2) * T:(mt % 2) * T + T]
                    ssum = ssum_tiles.pop((bh, mt))
                    nc.vector.reciprocal(out=ssum, in_=ssum)
                    nc.vector.tensor_scalar_mul(out=half, in0=half, scalar1=ssum)
                    if mt % 2 == 1:
                        at4 = at_tiles[bh]
                        q = mt // 2
                        nc.sync.dma_start_transpose(
                            out=at4[:, 2 * q:2 * q + 2, :, :], in_=abf2)
                        del ab_tiles[(bh, q)]

                def stage_d(bh):
                    b, h = divmod(bh, H)
                    at4 = at_tiles[bh]
                    avps = [psAV.tile([64, 512], F32, tag="av", name=f"avp{mc}")
                            for mc in range(2)]
                    for nt in range(T // P):
                        for mc in range(2):
                            nc.tensor.matmul(
                                avps[mc],
                                lhsT=v_sb[:, b * (T // P) + nt, h * HS:(h + 1) * HS],
                                rhs=at4[:, 4 * mc:4 * mc + 4, nt, :],
                                start=(nt == 0), stop=(nt == T // P - 1))
                    hh, po = h // 2, (h % 2) * 64
                    for mc in range(2):
                        nc.vector.tensor_copy(
                            out=avT[hh][po:po + 64,
                                        b * T + mc * 512:b * T + (mc + 1) * 512],
                            in_=avps[mc])
                    del at_tiles[bh]

                NMT8 = T // P
                for step in range(NBH + 2):
                    if 0 <= step - 1 < NBH:
                        at_tiles[step - 1] = atp.tile([P, NMT8, NMT8, P], BF16,
                                                      tag="at", name="at")
                    for mt in range(NMT8):
                        if step < NBH:
                            stage_a(step, mt)
                        if 0 <= step - 1 < NBH:
                            stage_bc_mm(step - 1, mt)
                            if mt > 0:
                                stage_bc_fin(step - 1, mt - 1)
                    if 0 <= step - 1 < NBH:
                        stage_bc_fin(step - 1, NMT8 - 1)
                    # stage_d last: at4(step-2)'s final xbar landed early in
                    # this step, so these matmuls are ready by the time the
                    # in-order PE queue reaches them
                    if step - 2 >= 0:
                        stage_d(step - 2)

            # ---------------- phase 4: out-proj + residual ----------------
            with tc.tile_pool(name="ph4", bufs=3) as sb4, \
                 tc.tile_pool(name="ps4", bufs=2, space="PSUM") as ps4:
                for mt in range(NMT):
                    op = ps4.tile([P, DM], F32, tag="op")
                    for sc in range(2):
                        nc.tensor.matmul(op,
                                         lhsT=avT[sc][:, mt * P:(mt + 1) * P],
                                         rhs=wo_sb[sc],
                                         start=(sc == 0), stop=(sc == 1))
                    ot = sb4.tile([P, DM], F32, tag="ot")
                    nc.vector.scalar_tensor_tensor(out=ot, in0=op, scalar=0.0,
                                                   in1=x_res[:, mt, :],
                                                   op0=mybir.AluOpType.bypass,
                                                   op1=mybir.AluOpType.add)
                    nc.vector.tensor_tensor(out=ot, in0=ot, in1=bo_b,
                                            op=mybir.AluOpType.add)
                    nc.sync.dma_start(out=out[mt * P:(mt + 1) * P, :], in_=ot)
    nc.finalize()
    return nc


_NC = None


def make_in_maps(inputs):
    f = lambda a: np.ascontiguousarray(np.asarray(a, dtype=np.float32))
    x = f(inputs["inputs"]).reshape(B, T, DM)
    pos = f(inputs["pos_enc"]).reshape(B, T, DM)
    wq0 = f(inputs["Wq"]).reshape(DM, DM)
    wk0 = f(inputs["Wk"]).reshape(DM, DM)
    wv0 = f(inputs["Wv"]).reshape(DM, DM)
    wp = f(inputs["Wp"]).reshape(DM, DM)
    wo = f(inputs["Wo"]).reshape(DM, DM)
    gamma = f(inputs["gamma"]).reshape(DM, 1)
    beta = f(inputs["beta"]).reshape(DM)
    # fold LN's gamma into the x-side weights, beta into the projection biases,
    # and bv through softmax (rows sum to 1) into the output bias
    wq, wk, wv = gamma * wq0, gamma * wk0, gamma * wv0
    bqu = (f(inputs["bq"]).reshape(DM) + f(inputs["pos_bias_u"]).reshape(DM)
           + beta @ wq0)
    bqv = (f(inputs["bq"]).reshape(DM) + f(inputs["pos_bias_v"]).reshape(DM)
           + beta @ wq0)
    bk = f(inputs["bk"]).reshape(DM) + beta @ wk0
    bv_eff = f(inputs["bv"]).reshape(DM) + beta @ wv0
    bo = f(inputs["bo"]) + bv_eff @ wo
    shared = dict(
        wq=wq, wk=wk, wv=wv, wp=wp, wo=wo,
        bqu=bqu, bqv=bqv, bk=bk, bo=bo,
    )
    in_maps = []
    for c in range(NCORES):
        sl = slice(c * BL, (c + 1) * BL)
        in_maps.append(dict(
            x=np.ascontiguousarray(x[sl].reshape(M, DM)),
            pos=np.ascontiguousarray(pos[sl].reshape(M, DM)),
            **shared,
        ))
    return in_maps


def kernel(**inputs) -> np.ndarray:
    global _NC
    if _NC is None:
        _NC = build_bass()
    in_maps = make_in_maps(inputs)
    res = run_bass_kernel_spmd(_NC, in_maps, core_ids=list(range(NCORES)))
    outs = [r["out"].reshape(BL, T, DM) for r in res.results]
    return np.concatenate(outs, axis=0)
